# revision 15
# baseline (speedup 1.0000x reference)
"""DualQDeformableAttention Trainium2 kernel (v2).

Sharding: 8 cores = 4 batch elements x 2 query-halves. Each core computes the
full dual-branch deformable attention for its 8192 queries of its batch
element (building the full value table for that batch) and emits
out[b][:, half] in C-major layout.

Per-core pipeline:
  P1: v = x3f @ Wv with a second matmul over a one-column-shifted window ->
      cell-major quad table qtab[cell][h][yy*2+xx][d] (bf16) assembled in
      SBUF (4 strided engine copies per row) and written with fully
      contiguous 256KB DMAs.
  P2: offset/attention-weight matmuls -> per-sample bilinear slot weights
      (softmax folded in, bf16) and f32 cell indices; a PE-transpose fold
      chain rearranges indices into the SWDGE wrapped-16 layout (no tiny-
      descriptor DMA), staged to DRAM in 4KB-run writes.
  P3: per (chunk, branch, head): SWDGE dma_gather with elem_step (per-head
      byte offset into the cell-major table; one 256B descriptor fetches all
      4 bilinear corners) -> bf16 weighted multiply -> strided 16:1 reduce ->
      outcat[n, (br,h,d)].  Gathers rotate across 4 SWDGE queues so
      descriptor generation runs on all four Q7 core pairs.
  P4: PE-transpose outcat, final matmul in Wout^T orientation -> C-major out.
"""

import numpy as np

HEADS = 8
NPOINTS = 4
C = 256
HH = 128
WW = 128
N = HH * WW          # 16384 cells per batch
NQ = N // 2          # 8192 queries per core
D = C // HEADS       # 32
QROW = 4 * D         # 128 elems: 4 corners x 32d per (cell, head)
CELL = HEADS * QROW  # 1024 elems per cell block
NT = NQ // 128       # 64 n-tiles per core
SLAB = 16            # n-tiles per weight-compute slab
CHUNK_NB = 8         # n-tiles per gather/blend chunk
NCHUNK = NT // CHUNK_NB   # 8
GPC = CHUNK_NB * NPOINTS  # gather groups per (chunk,br,h) = 32
IDXPC = GPC * 128         # indices per (chunk,br,h) = 4096

_PROGRAM = None
LAST_RESULT = None


def _build_program(stage=4):
    import os
    k3max = int(os.environ.get('K3CALLS', '999'))
    nqueues = int(os.environ.get('KQUEUES', '4'))
    scratch = int(os.environ.get('KSCRATCH', '32768'))
    _k3 = [0]
    import concourse.bass as bass
    import concourse.mybir as mybir
    from concourse import bacc
    from concourse.tile import TileContext
    from concourse.masks import make_identity

    dt = mybir.dt
    Alu = mybir.AluOpType
    AF = mybir.ActivationFunctionType
    AP = bass.AP
    X = mybir.AxisListType.X

    nc = bacc.Bacc('TRN2', num_swdge_queues=nqueues,
                   dynamic_dma_scratch_size=scratch)

    x1h = nc.dram_tensor('x1h', [C, NQ], dt.float32, kind='ExternalInput')
    x2h = nc.dram_tensor('x2h', [C, NQ], dt.float32, kind='ExternalInput')
    x3f = nc.dram_tensor('x3f', [C, N], dt.float32, kind='ExternalInput')
    wv_d = nc.dram_tensor('wv', [C, C], dt.float32, kind='ExternalInput')
    wcat_d = nc.dram_tensor('wcat', [C, 192], dt.float32, kind='ExternalInput')
    bcat_d = nc.dram_tensor('bcat', [192], dt.float32, kind='ExternalInput')
    wout_d = nc.dram_tensor('wout', [2 * C, C], dt.float32, kind='ExternalInput')
    bout_d = nc.dram_tensor('bout', [C], dt.float32, kind='ExternalInput')
    nx_d = nc.dram_tensor('nx', [128], dt.float32, kind='ExternalInput')
    ny_d = nc.dram_tensor('ny', [NT], dt.float32, kind='ExternalInput')
    out_d = nc.dram_tensor('out', [C, NQ], dt.float32, kind='ExternalOutput')

    W16_BR = HEADS * NT * NPOINTS * 4
    W16_H = NT * NPOINTS * 4

    with TileContext(nc) as tc:
        with tc.tile_pool(name='dram', bufs=1, space='DRAM') as dpool, \
             tc.tile_pool(name='consts', bufs=1) as cpool:

            qtab = dpool.tile([N, CELL], dt.bfloat16)
            # stag[ch][s][br][h][256]: wrapped-16 idx staging
            stag = dpool.tile([NCHUNK, 16, 2, HEADS, 256], dt.int16)

            # --- constants ---
            wv_sb = cpool.tile([128, 2, C], dt.float32)
            nc.sync.dma_start(wv_sb[:], wv_d[:].rearrange("(a p) n -> p a n", p=128))
            wcat_sb = cpool.tile([128, 2, 192], dt.float32)
            nc.sync.dma_start(wcat_sb[:], wcat_d[:].rearrange("(a p) n -> p a n", p=128))
            bcat_sb = cpool.tile([128, 192], dt.float32)
            nc.sync.dma_start(bcat_sb[:], AP(tensor=bcat_d, offset=0, ap=[[0, 128], [1, 192]]))
            wout_sb = cpool.tile([128, 4, C], dt.float32)
            nc.sync.dma_start(wout_sb[:], wout_d[:].rearrange("(a p) n -> p a n", p=128))
            bout_sb = cpool.tile([128, 2], dt.float32)
            nc.sync.dma_start(bout_sb[:], bout_d[:].rearrange("(a p) -> p a", p=128))
            nx_sb = cpool.tile([128, 1], dt.float32)
            nc.sync.dma_start(nx_sb[:], nx_d[:].rearrange("(a p) -> p a", p=128))
            ny_sb = cpool.tile([128, NT], dt.float32)
            nc.sync.dma_start(ny_sb[:], AP(tensor=ny_d, offset=0, ap=[[0, 128], [1, NT]]))
            ident = cpool.tile([128, 128], dt.float32)
            make_identity(nc, ident[:])
            w16 = cpool.tile([128, 2, HEADS, NT, NPOINTS, 4], dt.bfloat16)

            # ---------------- P1: cell-major quad table ----------------
            with tc.tile_pool(name='p1x', bufs=3) as p1x, \
                 tc.tile_pool(name='p1ps', bufs=3, space='PSUM') as p1ps, \
                 tc.tile_pool(name='p1q', bufs=3) as p1q:

                def qrow_write(t, vp0, vs0, vp1, vs1):
                    qrow = p1q.tile([128, CELL], dt.bfloat16, tag='qr', name='qr')
                    for slot, src in ((0, vp0), (1, vs0), (2, vp1), (3, vs1)):
                        dst = AP(tensor=qrow.tensor, offset=qrow.offset + slot * D,
                                 ap=[qrow.ap[0], [QROW, HEADS], [1, D]])
                        srca = AP(tensor=src.tensor, offset=src.offset,
                                  ap=[src.ap[0], [D, HEADS], [1, D]])
                        if slot < 2:
                            nc.vector.tensor_copy(dst, srca)
                        else:
                            nc.scalar.activation(dst, srca, AF.Copy)
                    nc.sync.dma_start(qtab[t * 128:(t + 1) * 128, :], qrow[:])

                prev = None
                for t in range(HH):
                    xa = p1x.tile([128, 129], dt.float32, tag='xa', name='xa')
                    xb = p1x.tile([128, 129], dt.float32, tag='xb', name='xb')
                    c0 = t * 128
                    if t < HH - 1:
                        nc.sync.dma_start(xa[:], x3f[0:128, c0:c0 + 129])
                        nc.sync.dma_start(xb[:], x3f[128:256, c0:c0 + 129])
                    else:
                        nc.sync.dma_start(xa[:, 0:128], x3f[0:128, c0:c0 + 128])
                        nc.sync.dma_start(xb[:, 0:128], x3f[128:256, c0:c0 + 128])
                        nc.sync.dma_start(xa[:, 128:129], x3f[0:128, N - 1:N])
                        nc.sync.dma_start(xb[:, 128:129], x3f[128:256, N - 1:N])
                    vp = p1ps.tile([128, C], dt.float32, tag='vp', name='vp')
                    nc.tensor.matmul(vp[:], xa[:, 0:128], wv_sb[:, 0], start=True, stop=False)
                    nc.tensor.matmul(vp[:], xb[:, 0:128], wv_sb[:, 1], start=False, stop=True)
                    vs = p1ps.tile([128, C], dt.float32, tag='vs', name='vs')
                    nc.tensor.matmul(vs[:], xa[:, 1:129], wv_sb[:, 0], start=True, stop=False)
                    nc.tensor.matmul(vs[:], xb[:, 1:129], wv_sb[:, 1], start=False, stop=True)
                    if prev is not None:
                        qrow_write(t - 1, prev[0], prev[1], vp, vs)
                    prev = (vp, vs)
                qrow_write(HH - 1, prev[0], prev[1], prev[0], prev[1])

            # ---------------- P2: offsets / weights / wrapped indices ----------------
            with tc.tile_pool(name='p2x', bufs=2) as p2x, \
                 tc.tile_pool(name='p2ps', bufs=2, space='PSUM') as p2ps, \
                 tc.tile_pool(name='p2s', bufs=2) as p2s, \
                 tc.tile_pool(name='p2t', bufs=1) as p2t, \
                 tc.tile_pool(name='p2w', bufs=1) as p2w, \
                 tc.tile_pool(name='fps', bufs=2, space='PSUM') as fps, \
                 tc.tile_pool(name='fsb', bufs=2) as fsb:
                for sl in range(NT // SLAB if stage >= 2 else 0):
                    oslab = p2s.tile([128, SLAB, 192], dt.float32, name='oslab')
                    xs = []
                    for j in range(SLAB):
                        if j % 4 == 0:
                            xs = []
                            cb = sl * SLAB * 128 + (j // 4) * 512
                            for i, (src, r0, r1) in enumerate([(x1h, 0, 128), (x1h, 128, 256),
                                                               (x2h, 0, 128), (x2h, 128, 256)]):
                                xt = p2x.tile([128, 512], dt.float32, tag=f'x{i}', name=f'x{i}')
                                nc.sync.dma_start(xt[:], src[r0:r1, cb:cb + 512])
                                xs.append(xt)
                        cs = slice((j % 4) * 128, (j % 4) * 128 + 128)
                        ops = p2ps.tile([128, 192], dt.float32, tag='ops', name='ops')
                        nc.tensor.matmul(ops[:, 0:96], xs[0][:, cs], wcat_sb[:, 0, 0:96], start=True, stop=False)
                        nc.tensor.matmul(ops[:, 0:96], xs[1][:, cs], wcat_sb[:, 1, 0:96], start=False, stop=True)
                        nc.tensor.matmul(ops[:, 96:192], xs[2][:, cs], wcat_sb[:, 0, 96:192], start=True, stop=False)
                        nc.tensor.matmul(ops[:, 96:192], xs[3][:, cs], wcat_sb[:, 1, 96:192], start=False, stop=True)
                        nc.vector.tensor_tensor(out=oslab[:, j], in0=ops[:], in1=bcat_sb[:], op=Alu.add)

                    idxw = p2w.tile([16, 2, 2, HEADS, 256], dt.int16, tag='idxw', name='idxw')
                    for br in range(2):
                        base = br * 96

                        def tl(tag, shape=None, dtp=dt.float32):
                            return p2t.tile(shape or [128, SLAB, 32], dtp, tag=tag, name=tag)

                        # softmax over the 32 (h,p) logits, per query
                        esl = tl('esl')
                        aw_in = AP(tensor=oslab.tensor, offset=oslab.offset + base + 64,
                                   ap=[oslab.ap[0], [192, SLAB], [1, 32]])
                        nc.scalar.activation(esl[:], aw_in, AF.Exp)
                        ssum = tl('ssum', [128, SLAB])
                        nc.vector.tensor_reduce(op=Alu.add, out=ssum[:], in_=esl[:], axis=X)
                        sinv = tl('sinv', [128, SLAB])
                        nc.vector.reciprocal(sinv[:], ssum[:])
                        es = tl('es')
                        nc.vector.tensor_tensor(
                            out=es[:], in0=esl[:],
                            in1=AP(tensor=sinv.tensor, offset=sinv.offset,
                                   ap=[sinv.ap[0], [1, SLAB], [0, 32]]),
                            op=Alu.mult)

                        # pixel coords: ix = off_x*128 + nx ; iy = off_y*128 + ny[t]
                        offx = AP(tensor=oslab.tensor, offset=oslab.offset + base,
                                  ap=[oslab.ap[0], [192, SLAB], [2, 32]])
                        offy = AP(tensor=oslab.tensor, offset=oslab.offset + base + 1,
                                  ap=[oslab.ap[0], [192, SLAB], [2, 32]])
                        ix = tl('ix')
                        nc.vector.scalar_tensor_tensor(
                            out=ix[:], in0=offx, scalar=128.0,
                            in1=AP(tensor=nx_sb.tensor, offset=nx_sb.offset,
                                   ap=[nx_sb.ap[0], [0, SLAB], [0, 32]]),
                            op0=Alu.mult, op1=Alu.add)
                        iy = tl('iy')
                        nc.vector.scalar_tensor_tensor(
                            out=iy[:], in0=offy, scalar=128.0,
                            in1=AP(tensor=ny_sb.tensor, offset=ny_sb.offset + sl * SLAB,
                                   ap=[ny_sb.ap[0], [1, SLAB], [0, 32]]),
                            op0=Alu.mult, op1=Alu.add)

                        def floorfrac(coord, pfx):
                            # floor via round(x - 0.5): f32->i32 convert rounds
                            half = tl(pfx + 'h')
                            nc.vector.tensor_scalar(out=half[:], in0=coord[:],
                                                    scalar1=-0.5, scalar2=None, op0=Alu.add)
                            ci = tl(pfx + 'i', dtp=dt.int32)
                            nc.vector.tensor_copy(ci[:], half[:])
                            cf = tl(pfx + 'f')
                            nc.vector.tensor_copy(cf[:], ci[:])
                            fr = tl(pfx + 'r')
                            nc.vector.tensor_tensor(out=fr[:], in0=coord[:], in1=cf[:], op=Alu.subtract)
                            return cf, fr

                        x0f, fx = floorfrac(ix, 'fx')
                        y0f, fy = floorfrac(iy, 'fy')

                        def slotw(c0f, fr, pfx):
                            # s0 = (1-f)*[0<=c0<=126] + f*[c0==-1]
                            # s1 = f*[0<=c0<=126] + (1-f)*[c0==127]
                            ge = tl(pfx + 'ge')
                            nc.vector.tensor_scalar(out=ge[:], in0=c0f[:], scalar1=0.0, scalar2=None, op0=Alu.is_ge)
                            le = tl(pfx + 'le')
                            nc.vector.tensor_scalar(out=le[:], in0=c0f[:], scalar1=126.0, scalar2=None, op0=Alu.is_le)
                            ax = tl(pfx + 'ax')
                            nc.vector.tensor_tensor(out=ax[:], in0=ge[:], in1=le[:], op=Alu.mult)
                            blo = tl(pfx + 'blo')
                            nc.vector.tensor_scalar(out=blo[:], in0=c0f[:], scalar1=-1.0, scalar2=None, op0=Alu.is_equal)
                            bhi = tl(pfx + 'bhi')
                            nc.vector.tensor_scalar(out=bhi[:], in0=c0f[:], scalar1=127.0, scalar2=None, op0=Alu.is_equal)
                            omf = tl(pfx + 'omf')
                            nc.vector.tensor_scalar(out=omf[:], in0=fr[:], scalar1=-1.0, scalar2=1.0, op0=Alu.mult, op1=Alu.add)
                            s0 = tl(pfx + 's0')
                            nc.vector.tensor_tensor(out=s0[:], in0=omf[:], in1=ax[:], op=Alu.mult)
                            t0 = tl(pfx + 't0')
                            nc.vector.tensor_tensor(out=t0[:], in0=fr[:], in1=blo[:], op=Alu.mult)
                            nc.vector.tensor_tensor(out=s0[:], in0=s0[:], in1=t0[:], op=Alu.add)
                            s1 = tl(pfx + 's1')
                            nc.vector.tensor_tensor(out=s1[:], in0=fr[:], in1=ax[:], op=Alu.mult)
                            t1 = tl(pfx + 't1')
                            nc.vector.tensor_tensor(out=t1[:], in0=omf[:], in1=bhi[:], op=Alu.mult)
                            nc.vector.tensor_tensor(out=s1[:], in0=s1[:], in1=t1[:], op=Alu.add)
                            return s0, s1

                        sx0, sx1 = slotw(x0f, fx, 'sx')
                        sy0, sy1 = slotw(y0f, fy, 'sy')

                        ay0 = tl('ay0')
                        nc.vector.tensor_tensor(out=ay0[:], in0=sy0[:], in1=es[:], op=Alu.mult)
                        ay1 = tl('ay1')
                        nc.vector.tensor_tensor(out=ay1[:], in0=sy1[:], in1=es[:], op=Alu.mult)

                        for (qi, ayv, sxv) in ((0, ay0, sx0), (1, ay0, sx1),
                                               (2, ay1, sx0), (3, ay1, sx1)):
                            wdst = AP(tensor=w16.tensor,
                                      offset=w16.offset + br * W16_BR + sl * SLAB * (NPOINTS * 4) + qi,
                                      ap=[w16.ap[0], [NPOINTS * 4, SLAB], [W16_H, HEADS], [4, NPOINTS]])
                            win0 = AP(tensor=ayv.tensor, offset=ayv.offset,
                                      ap=[ayv.ap[0], [32, SLAB], [NPOINTS, HEADS], [1, NPOINTS]])
                            win1 = AP(tensor=sxv.tensor, offset=sxv.offset,
                                      ap=[sxv.ap[0], [32, SLAB], [NPOINTS, HEADS], [1, NPOINTS]])
                            nc.vector.tensor_tensor(out=wdst, in0=win0, in1=win1, op=Alu.mult)

                        xbc = tl('xb')
                        nc.vector.tensor_scalar(out=xbc[:], in0=x0f[:], scalar1=0.0, scalar2=126.0, op0=Alu.max, op1=Alu.min)
                        ybc = tl('yb')
                        nc.vector.tensor_scalar(out=ybc[:], in0=y0f[:], scalar1=0.0, scalar2=126.0, op0=Alu.max, op1=Alu.min)
                        idxf = tl('idxf')
                        nc.vector.scalar_tensor_tensor(out=idxf[:], in0=ybc[:], scalar=128.0,
                                                       in1=xbc[:], op0=Alu.mult, op1=Alu.add)
                        # idxf in h-major layout [128][h][nt][p]
                        idxft = tl('idxft', [128, HEADS, SLAB, NPOINTS])
                        nc.vector.tensor_copy(
                            idxft[:],
                            AP(tensor=idxf.tensor, offset=idxf.offset,
                               ap=[idxf.ap[0], [NPOINTS, HEADS], [32, SLAB], [1, NPOINTS]]))

                        # PE fold: [128 q][32 (nt,p)] -> wrapped [16][(g,j)]
                        for c in range(2):
                            for h in range(HEADS):
                                tp1 = fps.tile([32, 128], dt.float32, tag='tp1', name='tp1')
                                nc.tensor.transpose(tp1[:], idxft[:, h, c * 8:(c + 1) * 8, :], ident[:])
                                ts1 = fsb.tile([32, 128], dt.float32, tag='ts1', name='ts1')
                                nc.scalar.activation(ts1[:], tp1[:], AF.Copy)
                                for jj in range(8):
                                    u = fps.tile([16, 32], dt.float32, tag='u', name='u')
                                    nc.tensor.transpose(u[:], ts1[:, 16 * jj:16 * jj + 16], ident[0:32, 0:32])
                                    udst = AP(tensor=idxw.tensor,
                                              offset=idxw.offset + ((c * 2 + br) * HEADS + h) * 256 + jj,
                                              ap=[idxw.ap[0], [8, 32]])
                                    nc.vector.tensor_copy(udst, u[:])

                    for c in range(2):
                        ch = sl * 2 + c
                        nc.sync.dma_start(stag[ch], idxw[:, c])

            # ---------------- P3 + P4 ----------------
            with tc.tile_pool(name='p3i', bufs=2) as p3i, \
                 tc.tile_pool(name='p3g', bufs=4) as p3g, \
                 tc.tile_pool(name='p3w', bufs=2) as p3w, \
                 tc.tile_pool(name='p3o', bufs=2) as p3o, \
                 tc.tile_pool(name='p4ps', bufs=4, space='PSUM') as p4ps, \
                 tc.tile_pool(name='p4t', bufs=1) as p4t, \
                 tc.tile_pool(name='p4f', bufs=2, space='PSUM') as p4f, \
                 tc.tile_pool(name='p4o', bufs=2) as p4o:
                for ch in range(NCHUNK if stage >= 3 else 0):
                    idxt = p3i.tile([128, 2 * HEADS * 256], dt.int16, tag='idxt', name='idxt')
                    ssrc = AP(tensor=stag.tensor, offset=stag.offset + ch * (16 * 4096),
                              ap=[[0, 8], [4096, 16], [1, 4096]])
                    nc.sync.dma_start(idxt[:], ssrc)
                    outcat = p3o.tile([128, CHUNK_NB, 512], dt.float32, name='outcat')
                    for br in range(2):
                        for h in range(HEADS):
                            if _k3[0] >= k3max:
                                continue
                            _k3[0] += 1
                            gat = p3g.tile([128, GPC, QROW], dt.bfloat16, tag='gat', name='gat')
                            qv = AP(tensor=qtab.tensor, offset=qtab.offset + h * QROW,
                                    ap=[[CELL, N], [1, QROW]])
                            ib = (br * HEADS + h) * 256
                            for sub in range(IDXPC // 1024):
                                nc.gpsimd.dma_gather(
                                    out_ap=gat[:, sub * 8:(sub + 1) * 8, :],
                                    in_ap=qv,
                                    idxs_ap=idxt[:, ib + sub * 64:ib + (sub + 1) * 64],
                                    num_idxs=1024, num_idxs_reg=1024,
                                    elem_size=QROW, elem_step=CELL,
                                    queue_num=(br * HEADS + h) % nqueues)
                            wg = p3w.tile([128, GPC, QROW], dt.bfloat16, tag='wg', name='wg')
                            win = AP(tensor=w16.tensor,
                                     offset=w16.offset + br * W16_BR + h * W16_H
                                     + ch * CHUNK_NB * NPOINTS * 4,
                                     ap=[w16.ap[0], [4, GPC], [1, 4], [0, D]])
                            nc.vector.tensor_tensor(
                                out=wg[:].rearrange("p g (q d) -> p g q d", d=D),
                                in0=gat[:].rearrange("p g (q d) -> p g q d", d=D),
                                in1=win, op=Alu.mult)
                            rin = AP(tensor=wg.tensor, offset=wg.offset,
                                     ap=[wg.ap[0], [NPOINTS * QROW, CHUNK_NB], [1, D], [D, 16]])
                            rout = AP(tensor=outcat.tensor,
                                      offset=outcat.offset + br * 256 + h * D,
                                      ap=[outcat.ap[0], [512, CHUNK_NB], [1, D]])
                            nc.vector.tensor_reduce(op=Alu.add, out=rout, in_=rin, axis=X)

                    if stage < 4:
                        continue
                    ocT = p4t.tile([128, 4, CHUNK_NB * 128], dt.float32, name='ocT')
                    for nb in range(CHUNK_NB):
                        for k in range(4):
                            tp = p4ps.tile([128, 128], dt.float32, tag='tp', name='tp')
                            nc.tensor.transpose(tp[:], outcat[:, nb, k * 128:(k + 1) * 128], ident[:])
                            nc.scalar.activation(ocT[:, k, nb * 128:(nb + 1) * 128], tp[:], AF.Copy)
                    for chl in range(2):
                        for wnd in range(CHUNK_NB * 128 // 512):
                            fp = p4f.tile([128, 512], dt.float32, tag='fp', name='fp')
                            for k in range(4):
                                nc.tensor.matmul(
                                    fp[:], wout_sb[:, k, chl * 128:(chl + 1) * 128],
                                    ocT[:, k, wnd * 512:(wnd + 1) * 512],
                                    start=(k == 0), stop=(k == 3))
                            ob = p4o.tile([128, 512], dt.float32, tag='ob', name='ob')
                            nc.scalar.activation(ob[:], fp[:], AF.Identity, bias=bout_sb[:, chl:chl + 1])
                            col0 = ch * (CHUNK_NB * 128) + wnd * 512
                            nc.sync.dma_start(out_d[chl * 128:(chl + 1) * 128, col0:col0 + 512], ob[:])

    nc.compile()
    return nc


def _get_program():
    global _PROGRAM
    if _PROGRAM is None:
        import os
        _PROGRAM = _build_program(stage=int(os.environ.get('KSTAGE', '4')))
    return _PROGRAM


def kernel(x1, x2, x3, Wv, Woff1, boff1, Woff2, boff2, Waw1, baw1, Waw2, baw2, Wout, bout):
    from concourse.bass_utils import run_bass_kernel_spmd

    x1 = np.asarray(x1, dtype=np.float32)
    x2 = np.asarray(x2, dtype=np.float32)
    x3 = np.asarray(x3, dtype=np.float32)
    B = x1.shape[0]
    wcat = np.ascontiguousarray(np.concatenate(
        [np.asarray(Woff1), np.asarray(Waw1), np.asarray(Woff2), np.asarray(Waw2)],
        axis=1).astype(np.float32))
    bcat = np.ascontiguousarray(np.concatenate(
        [np.asarray(boff1), np.asarray(baw1), np.asarray(boff2), np.asarray(baw2)]
    ).astype(np.float32))
    wv = np.ascontiguousarray(np.asarray(Wv, dtype=np.float32))
    wout = np.ascontiguousarray(np.asarray(Wout, dtype=np.float32))
    boutv = np.ascontiguousarray(np.asarray(bout, dtype=np.float32))
    nx = np.arange(128, dtype=np.float32)

    nc = _get_program()
    in_maps = []
    for core in range(8):
        b, half = core // 2, core % 2
        x1f = x1[b].reshape(C, N)
        x2f = x2[b].reshape(C, N)
        in_maps.append({
            'x1h': np.ascontiguousarray(x1f[:, half * NQ:(half + 1) * NQ]),
            'x2h': np.ascontiguousarray(x2f[:, half * NQ:(half + 1) * NQ]),
            'x3f': np.ascontiguousarray(x3[b].reshape(C, N)),
            'wv': wv, 'wcat': wcat, 'bcat': bcat,
            'wout': wout, 'bout': boutv,
            'nx': nx,
            'ny': (half * NT + np.arange(NT)).astype(np.float32),
        })
    import os as _os
    ncores = int(_os.environ.get('NCORES', '8'))
    res = run_bass_kernel_spmd(nc, in_maps[:ncores], core_ids=list(range(ncores)))
    global LAST_RESULT
    LAST_RESULT = res
    out = np.zeros((B, C, HH, WW), dtype=np.float32)
    for core in range(ncores):
        b, half = core // 2, core % 2
        out[b].reshape(C, N)[:, half * NQ:(half + 1) * NQ] = res.results[core]['out']
    return out


# revision 21
# speedup vs baseline: 1.1773x; 1.1773x over previous
"""DualQDeformableAttention Trainium2 kernel (v2).

Sharding: 8 cores = 4 batch elements x 2 query-halves. Each core computes the
full dual-branch deformable attention for its 8192 queries of its batch
element (building the full value table for that batch) and emits
out[b][:, half] in C-major layout.

Per-core pipeline:
  P1: v = x3f @ Wv with a second matmul over a one-column-shifted window ->
      cell-major quad table qtab[cell][h][yy*2+xx][d] (bf16) assembled in
      SBUF (4 strided engine copies per row) and written with fully
      contiguous 256KB DMAs.
  P2: offset/attention-weight matmuls -> per-sample bilinear slot weights
      (softmax folded in, bf16) and f32 cell indices; a PE-transpose fold
      chain rearranges indices into the SWDGE wrapped-16 layout (no tiny-
      descriptor DMA), staged to DRAM in 4KB-run writes.
  P3: per (chunk, branch, head): SWDGE dma_gather with elem_step (per-head
      byte offset into the cell-major table; one 256B descriptor fetches all
      4 bilinear corners) -> bf16 weighted multiply -> strided 16:1 reduce ->
      outcat[n, (br,h,d)].  Gathers rotate across 4 SWDGE queues so
      descriptor generation runs on all four Q7 core pairs.
  P4: PE-transpose outcat, final matmul in Wout^T orientation -> C-major out.
"""

import numpy as np

HEADS = 8
NPOINTS = 4
C = 256
HH = 128
WW = 128
N = HH * WW          # 16384 cells per batch
NQ = N // 2          # 8192 queries per core
D = C // HEADS       # 32
QROW = 4 * D         # 128 elems: 4 corners x 32d per (cell, head)
CELL = HEADS * QROW  # 1024 elems per cell block
NT = NQ // 128       # 64 n-tiles per core
SLAB = 16            # n-tiles per weight-compute slab
CHUNK_NB = 8         # n-tiles per gather/blend chunk
NCHUNK = NT // CHUNK_NB   # 8
GPC = CHUNK_NB * NPOINTS  # gather groups per (chunk,br,h) = 32
IDXPC = GPC * 128         # indices per (chunk,br,h) = 4096

_PROGRAM = None
LAST_RESULT = None


def _build_program(stage=4):
    import os
    k3max = int(os.environ.get('K3CALLS', '999'))
    nqueues = int(os.environ.get('KQUEUES', '4'))
    scratch = int(os.environ.get('KSCRATCH', '32768'))
    _k3 = [0]
    import concourse.bass as bass
    import concourse.mybir as mybir
    from concourse import bacc
    from concourse.tile import TileContext
    from concourse.masks import make_identity

    dt = mybir.dt
    Alu = mybir.AluOpType
    AF = mybir.ActivationFunctionType
    AP = bass.AP
    X = mybir.AxisListType.X

    nc = bacc.Bacc('TRN2', num_swdge_queues=nqueues,
                   dynamic_dma_scratch_size=scratch)

    x1h = nc.dram_tensor('x1h', [C, NQ], dt.float32, kind='ExternalInput')
    x2h = nc.dram_tensor('x2h', [C, NQ], dt.float32, kind='ExternalInput')
    x3f = nc.dram_tensor('x3f', [C, N], dt.float32, kind='ExternalInput')
    wv_d = nc.dram_tensor('wv', [C, C], dt.float32, kind='ExternalInput')
    wcat_d = nc.dram_tensor('wcat', [C, 192], dt.float32, kind='ExternalInput')
    bcat_d = nc.dram_tensor('bcat', [192], dt.float32, kind='ExternalInput')
    wout_d = nc.dram_tensor('wout', [2 * C, C], dt.float32, kind='ExternalInput')
    bout_d = nc.dram_tensor('bout', [C], dt.float32, kind='ExternalInput')
    nx_d = nc.dram_tensor('nx', [128], dt.float32, kind='ExternalInput')
    ny_d = nc.dram_tensor('ny', [NT], dt.float32, kind='ExternalInput')
    out_d = nc.dram_tensor('out', [C, NQ], dt.float32, kind='ExternalOutput')

    W16_BR = HEADS * NT * NPOINTS * 4
    W16_H = NT * NPOINTS * 4

    with TileContext(nc) as tc:
        with tc.tile_pool(name='dram', bufs=1, space='DRAM') as dpool, \
             tc.tile_pool(name='consts', bufs=1) as cpool:

            qtab = dpool.tile([N, CELL], dt.bfloat16)
            # stag[ch][s][br][h][256]: wrapped-16 idx staging
            stag = dpool.tile([NCHUNK, 16, 2, HEADS, 256], dt.int16)

            # --- constants ---
            wv_sb = cpool.tile([128, 2, C], dt.float32)
            nc.sync.dma_start(wv_sb[:], wv_d[:].rearrange("(a p) n -> p a n", p=128))
            wcat_sb = cpool.tile([128, 2, 192], dt.float32)
            nc.sync.dma_start(wcat_sb[:], wcat_d[:].rearrange("(a p) n -> p a n", p=128))
            bcat_sb = cpool.tile([128, 192], dt.float32)
            nc.sync.dma_start(bcat_sb[:], AP(tensor=bcat_d, offset=0, ap=[[0, 128], [1, 192]]))
            wout_sb = cpool.tile([128, 4, C], dt.float32)
            nc.sync.dma_start(wout_sb[:], wout_d[:].rearrange("(a p) n -> p a n", p=128))
            bout_sb = cpool.tile([128, 2], dt.float32)
            nc.sync.dma_start(bout_sb[:], bout_d[:].rearrange("(a p) -> p a", p=128))
            nx_sb = cpool.tile([128, 1], dt.float32)
            nc.sync.dma_start(nx_sb[:], nx_d[:].rearrange("(a p) -> p a", p=128))
            ny_sb = cpool.tile([128, NT], dt.float32)
            nc.sync.dma_start(ny_sb[:], AP(tensor=ny_d, offset=0, ap=[[0, 128], [1, NT]]))
            ident = cpool.tile([128, 128], dt.float32)
            make_identity(nc, ident[:])
            identb = cpool.tile([128, 128], dt.bfloat16)
            nc.vector.tensor_copy(identb[:], ident[:])
            w16 = cpool.tile([128, 2, HEADS, NT, NPOINTS, 4], dt.bfloat16)

            # ---------------- P1: cell-major quad table ----------------
            with tc.tile_pool(name='p1x', bufs=3) as p1x, \
                 tc.tile_pool(name='p1ps', bufs=3, space='PSUM') as p1ps, \
                 tc.tile_pool(name='p1q', bufs=3) as p1q:

                def qrow_write(t, vp0, vs0, vp1, vs1):
                    qrow = p1q.tile([128, CELL], dt.bfloat16, tag='qr', name='qr')
                    for slot, src in ((0, vp0), (1, vs0), (2, vp1), (3, vs1)):
                        dst = AP(tensor=qrow.tensor, offset=qrow.offset + slot * D,
                                 ap=[qrow.ap[0], [QROW, HEADS], [1, D]])
                        srca = AP(tensor=src.tensor, offset=src.offset,
                                  ap=[src.ap[0], [D, HEADS], [1, D]])
                        if slot < 2:
                            nc.vector.tensor_copy(dst, srca)
                        else:
                            nc.scalar.activation(dst, srca, AF.Copy)
                    nc.sync.dma_start(qtab[t * 128:(t + 1) * 128, :], qrow[:])

                prev = None
                for t in range(HH):
                    xa = p1x.tile([128, 129], dt.float32, tag='xa', name='xa')
                    xb = p1x.tile([128, 129], dt.float32, tag='xb', name='xb')
                    c0 = t * 128
                    if t < HH - 1:
                        nc.sync.dma_start(xa[:], x3f[0:128, c0:c0 + 129])
                        nc.sync.dma_start(xb[:], x3f[128:256, c0:c0 + 129])
                    else:
                        nc.sync.dma_start(xa[:, 0:128], x3f[0:128, c0:c0 + 128])
                        nc.sync.dma_start(xb[:, 0:128], x3f[128:256, c0:c0 + 128])
                        nc.sync.dma_start(xa[:, 128:129], x3f[0:128, N - 1:N])
                        nc.sync.dma_start(xb[:, 128:129], x3f[128:256, N - 1:N])
                    vp = p1ps.tile([128, C], dt.float32, tag='vp', name='vp')
                    nc.tensor.matmul(vp[:], xa[:, 0:128], wv_sb[:, 0], start=True, stop=False)
                    nc.tensor.matmul(vp[:], xb[:, 0:128], wv_sb[:, 1], start=False, stop=True)
                    vs = p1ps.tile([128, C], dt.float32, tag='vs', name='vs')
                    nc.tensor.matmul(vs[:], xa[:, 1:129], wv_sb[:, 0], start=True, stop=False)
                    nc.tensor.matmul(vs[:], xb[:, 1:129], wv_sb[:, 1], start=False, stop=True)
                    if prev is not None:
                        qrow_write(t - 1, prev[0], prev[1], vp, vs)
                    prev = (vp, vs)
                qrow_write(HH - 1, prev[0], prev[1], prev[0], prev[1])

            # ---------------- P2: offsets / weights / wrapped indices ----------------
            with tc.tile_pool(name='p2x', bufs=2) as p2x, \
                 tc.tile_pool(name='p2ps', bufs=2, space='PSUM') as p2ps, \
                 tc.tile_pool(name='p2s', bufs=2) as p2s, \
                 tc.tile_pool(name='p2t', bufs=1) as p2t, \
                 tc.tile_pool(name='p2w', bufs=1) as p2w, \
                 tc.tile_pool(name='fps', bufs=2, space='PSUM') as fps, \
                 tc.tile_pool(name='fsb', bufs=2) as fsb:
                for sl in range(NT // SLAB if stage >= 2 else 0):
                    oslab = p2s.tile([128, SLAB, 192], dt.float32, name='oslab')
                    xs = []
                    for j in range(SLAB):
                        if j % 4 == 0:
                            xs = []
                            cb = sl * SLAB * 128 + (j // 4) * 512
                            for i, (src, r0, r1) in enumerate([(x1h, 0, 128), (x1h, 128, 256),
                                                               (x2h, 0, 128), (x2h, 128, 256)]):
                                xt = p2x.tile([128, 512], dt.float32, tag=f'x{i}', name=f'x{i}')
                                nc.sync.dma_start(xt[:], src[r0:r1, cb:cb + 512])
                                xs.append(xt)
                        cs = slice((j % 4) * 128, (j % 4) * 128 + 128)
                        ops = p2ps.tile([128, 192], dt.float32, tag='ops', name='ops')
                        nc.tensor.matmul(ops[:, 0:96], xs[0][:, cs], wcat_sb[:, 0, 0:96], start=True, stop=False)
                        nc.tensor.matmul(ops[:, 0:96], xs[1][:, cs], wcat_sb[:, 1, 0:96], start=False, stop=True)
                        nc.tensor.matmul(ops[:, 96:192], xs[2][:, cs], wcat_sb[:, 0, 96:192], start=True, stop=False)
                        nc.tensor.matmul(ops[:, 96:192], xs[3][:, cs], wcat_sb[:, 1, 96:192], start=False, stop=True)
                        nc.vector.tensor_tensor(out=oslab[:, j], in0=ops[:], in1=bcat_sb[:], op=Alu.add)

                    idxw = p2w.tile([16, 2, 2, HEADS, 256], dt.int16, tag='idxw', name='idxw')
                    for br in range(2):
                        base = br * 96

                        def tl(tag, shape=None, dtp=dt.float32):
                            return p2t.tile(shape or [128, SLAB, 32], dtp, tag=tag, name=tag)

                        # softmax over the 32 (h,p) logits, per query
                        esl = tl('esl')
                        aw_in = AP(tensor=oslab.tensor, offset=oslab.offset + base + 64,
                                   ap=[oslab.ap[0], [192, SLAB], [1, 32]])
                        nc.scalar.activation(esl[:], aw_in, AF.Exp)
                        ssum = tl('ssum', [128, SLAB])
                        nc.vector.tensor_reduce(op=Alu.add, out=ssum[:], in_=esl[:], axis=X)
                        sinv = tl('sinv', [128, SLAB])
                        nc.vector.reciprocal(sinv[:], ssum[:])
                        es = tl('es')
                        nc.vector.tensor_tensor(
                            out=es[:], in0=esl[:],
                            in1=AP(tensor=sinv.tensor, offset=sinv.offset,
                                   ap=[sinv.ap[0], [1, SLAB], [0, 32]]),
                            op=Alu.mult)

                        # pixel coords: ix = off_x*128 + nx ; iy = off_y*128 + ny[t]
                        offx = AP(tensor=oslab.tensor, offset=oslab.offset + base,
                                  ap=[oslab.ap[0], [192, SLAB], [2, 32]])
                        offy = AP(tensor=oslab.tensor, offset=oslab.offset + base + 1,
                                  ap=[oslab.ap[0], [192, SLAB], [2, 32]])
                        ix = tl('ix')
                        nc.vector.scalar_tensor_tensor(
                            out=ix[:], in0=offx, scalar=128.0,
                            in1=AP(tensor=nx_sb.tensor, offset=nx_sb.offset,
                                   ap=[nx_sb.ap[0], [0, SLAB], [0, 32]]),
                            op0=Alu.mult, op1=Alu.add)
                        iy = tl('iy')
                        nc.vector.scalar_tensor_tensor(
                            out=iy[:], in0=offy, scalar=128.0,
                            in1=AP(tensor=ny_sb.tensor, offset=ny_sb.offset + sl * SLAB,
                                   ap=[ny_sb.ap[0], [1, SLAB], [0, 32]]),
                            op0=Alu.mult, op1=Alu.add)

                        def floorfrac(coord, pfx):
                            # floor via round(x - 0.5): f32->i32 convert rounds
                            half = tl(pfx + 'h')
                            nc.vector.tensor_scalar(out=half[:], in0=coord[:],
                                                    scalar1=-0.5, scalar2=None, op0=Alu.add)
                            ci = tl(pfx + 'i', dtp=dt.int32)
                            nc.vector.tensor_copy(ci[:], half[:])
                            cf = tl(pfx + 'f')
                            nc.vector.tensor_copy(cf[:], ci[:])
                            fr = tl(pfx + 'r')
                            nc.vector.tensor_tensor(out=fr[:], in0=coord[:], in1=cf[:], op=Alu.subtract)
                            return cf, fr

                        x0f, fx = floorfrac(ix, 'fx')
                        y0f, fy = floorfrac(iy, 'fy')

                        def slotw(c0f, fr, pfx):
                            # s0 = (1-f)*[0<=c0<=126] + f*[c0==-1]
                            # s1 = f*[0<=c0<=126] + (1-f)*[c0==127]
                            ge = tl(pfx + 'ge')
                            nc.vector.tensor_scalar(out=ge[:], in0=c0f[:], scalar1=0.0, scalar2=None, op0=Alu.is_ge)
                            le = tl(pfx + 'le')
                            nc.vector.tensor_scalar(out=le[:], in0=c0f[:], scalar1=126.0, scalar2=None, op0=Alu.is_le)
                            ax = tl(pfx + 'ax')
                            nc.vector.tensor_tensor(out=ax[:], in0=ge[:], in1=le[:], op=Alu.mult)
                            blo = tl(pfx + 'blo')
                            nc.vector.tensor_scalar(out=blo[:], in0=c0f[:], scalar1=-1.0, scalar2=None, op0=Alu.is_equal)
                            bhi = tl(pfx + 'bhi')
                            nc.vector.tensor_scalar(out=bhi[:], in0=c0f[:], scalar1=127.0, scalar2=None, op0=Alu.is_equal)
                            omf = tl(pfx + 'omf')
                            nc.vector.tensor_scalar(out=omf[:], in0=fr[:], scalar1=-1.0, scalar2=1.0, op0=Alu.mult, op1=Alu.add)
                            s0 = tl(pfx + 's0')
                            nc.vector.tensor_tensor(out=s0[:], in0=omf[:], in1=ax[:], op=Alu.mult)
                            t0 = tl(pfx + 't0')
                            nc.vector.tensor_tensor(out=t0[:], in0=fr[:], in1=blo[:], op=Alu.mult)
                            nc.vector.tensor_tensor(out=s0[:], in0=s0[:], in1=t0[:], op=Alu.add)
                            s1 = tl(pfx + 's1')
                            nc.vector.tensor_tensor(out=s1[:], in0=fr[:], in1=ax[:], op=Alu.mult)
                            t1 = tl(pfx + 't1')
                            nc.vector.tensor_tensor(out=t1[:], in0=omf[:], in1=bhi[:], op=Alu.mult)
                            nc.vector.tensor_tensor(out=s1[:], in0=s1[:], in1=t1[:], op=Alu.add)
                            return s0, s1

                        sx0, sx1 = slotw(x0f, fx, 'sx')
                        sy0, sy1 = slotw(y0f, fy, 'sy')

                        ay0 = tl('ay0')
                        nc.vector.tensor_tensor(out=ay0[:], in0=sy0[:], in1=es[:], op=Alu.mult)
                        ay1 = tl('ay1')
                        nc.vector.tensor_tensor(out=ay1[:], in0=sy1[:], in1=es[:], op=Alu.mult)

                        for (qi, ayv, sxv) in ((0, ay0, sx0), (1, ay0, sx1),
                                               (2, ay1, sx0), (3, ay1, sx1)):
                            wdst = AP(tensor=w16.tensor,
                                      offset=w16.offset + br * W16_BR + sl * SLAB * (NPOINTS * 4) + qi,
                                      ap=[w16.ap[0], [NPOINTS * 4, SLAB], [W16_H, HEADS], [4, NPOINTS]])
                            win0 = AP(tensor=ayv.tensor, offset=ayv.offset,
                                      ap=[ayv.ap[0], [32, SLAB], [NPOINTS, HEADS], [1, NPOINTS]])
                            win1 = AP(tensor=sxv.tensor, offset=sxv.offset,
                                      ap=[sxv.ap[0], [32, SLAB], [NPOINTS, HEADS], [1, NPOINTS]])
                            nc.vector.tensor_tensor(out=wdst, in0=win0, in1=win1, op=Alu.mult)

                        xbc = tl('xb')
                        nc.vector.tensor_scalar(out=xbc[:], in0=x0f[:], scalar1=0.0, scalar2=126.0, op0=Alu.max, op1=Alu.min)
                        ybc = tl('yb')
                        nc.vector.tensor_scalar(out=ybc[:], in0=y0f[:], scalar1=0.0, scalar2=126.0, op0=Alu.max, op1=Alu.min)
                        idxf = tl('idxf')
                        nc.vector.scalar_tensor_tensor(out=idxf[:], in0=ybc[:], scalar=128.0,
                                                       in1=xbc[:], op0=Alu.mult, op1=Alu.add)
                        # idxf in h-major layout [128][h][nt][p]
                        idxft = tl('idxft', [128, HEADS, SLAB, NPOINTS])
                        nc.vector.tensor_copy(
                            idxft[:],
                            AP(tensor=idxf.tensor, offset=idxf.offset,
                               ap=[idxf.ap[0], [NPOINTS, HEADS], [32, SLAB], [1, NPOINTS]]))

                        # PE fold: [128 q][32 (nt,p)] -> wrapped [16][(g,j)]
                        for c in range(2):
                            for h in range(HEADS):
                                tp1 = fps.tile([32, 128], dt.float32, tag='tp1', name='tp1')
                                nc.tensor.transpose(tp1[:], idxft[:, h, c * 8:(c + 1) * 8, :], ident[:])
                                ts1 = fsb.tile([32, 128], dt.float32, tag='ts1', name='ts1')
                                nc.scalar.activation(ts1[:], tp1[:], AF.Copy)
                                for jj in range(8):
                                    u = fps.tile([16, 32], dt.float32, tag='u', name='u')
                                    nc.tensor.transpose(u[:], ts1[:, 16 * jj:16 * jj + 16], ident[0:32, 0:32])
                                    udst = AP(tensor=idxw.tensor,
                                              offset=idxw.offset + ((c * 2 + br) * HEADS + h) * 256 + jj,
                                              ap=[idxw.ap[0], [8, 32]])
                                    nc.vector.tensor_copy(udst, u[:])

                    for c in range(2):
                        ch = sl * 2 + c
                        nc.sync.dma_start(stag[ch], idxw[:, c])

            # ---------------- P3 + P4 ----------------
            with tc.tile_pool(name='p3i', bufs=2) as p3i, \
                 tc.tile_pool(name='p3g', bufs=6) as p3g, \
                 tc.tile_pool(name='p3w', bufs=3) as p3w, \
                 tc.tile_pool(name='p3o', bufs=2) as p3o, \
                 tc.tile_pool(name='p4ps', bufs=4, space='PSUM') as p4ps, \
                 tc.tile_pool(name='p4t', bufs=1) as p4t, \
                 tc.tile_pool(name='p4f', bufs=2, space='PSUM') as p4f, \
                 tc.tile_pool(name='p4o', bufs=2) as p4o:
                for ch in range(NCHUNK if stage >= 3 else 0):
                    idxt = p3i.tile([128, 2 * HEADS * 256], dt.int16, tag='idxt', name='idxt')
                    ssrc = AP(tensor=stag.tensor, offset=stag.offset + ch * (16 * 4096),
                              ap=[[0, 8], [4096, 16], [1, 4096]])
                    nc.sync.dma_start(idxt[:], ssrc)
                    outcat = p3o.tile([128, CHUNK_NB, 512], dt.bfloat16, name='outcat')
                    for br in range(2):
                        for h in range(HEADS):
                            if _k3[0] >= k3max:
                                continue
                            _k3[0] += 1
                            gat = p3g.tile([128, GPC, QROW], dt.bfloat16, tag='gat', name='gat')
                            qv = AP(tensor=qtab.tensor, offset=qtab.offset + h * QROW,
                                    ap=[[CELL, N], [1, QROW]])
                            ib = (br * HEADS + h) * 256
                            for sub in range(IDXPC // 1024):
                                nc.gpsimd.dma_gather(
                                    out_ap=gat[:, sub * 8:(sub + 1) * 8, :],
                                    in_ap=qv,
                                    idxs_ap=idxt[:, ib + sub * 64:ib + (sub + 1) * 64],
                                    num_idxs=1024, num_idxs_reg=1024,
                                    elem_size=QROW, elem_step=CELL,
                                    queue_num=(br * HEADS + h) % nqueues)
                            wg = p3w.tile([128, GPC, QROW], dt.bfloat16, tag='wg', name='wg')
                            win = AP(tensor=w16.tensor,
                                     offset=w16.offset + br * W16_BR + h * W16_H
                                     + ch * CHUNK_NB * NPOINTS * 4,
                                     ap=[w16.ap[0], [4, GPC], [1, 4], [0, D]])
                            nc.vector.tensor_tensor(
                                out=wg[:].rearrange("p g (q d) -> p g q d", d=D),
                                in0=gat[:].rearrange("p g (q d) -> p g q d", d=D),
                                in1=win, op=Alu.mult)
                            rin = AP(tensor=wg.tensor, offset=wg.offset,
                                     ap=[wg.ap[0], [NPOINTS * QROW, CHUNK_NB], [1, D], [D, 16]])
                            rout = AP(tensor=outcat.tensor,
                                      offset=outcat.offset + br * 256 + h * D,
                                      ap=[outcat.ap[0], [512, CHUNK_NB], [1, D]])
                            with nc.allow_low_precision(reason='16:1 corner sum fits bf16; 2e-2 gate'):
                                nc.vector.tensor_reduce(op=Alu.add, out=rout, in_=rin, axis=X)

                    if stage < 4:
                        continue
                    ocT = p4t.tile([128, 4, CHUNK_NB * 128], dt.float32, name='ocT')
                    for nb in range(CHUNK_NB):
                        for k in range(4):
                            tp = p4ps.tile([128, 128], dt.bfloat16, tag='tp', name='tp')
                            nc.tensor.transpose(tp[:], outcat[:, nb, k * 128:(k + 1) * 128], identb[:])
                            nc.scalar.activation(ocT[:, k, nb * 128:(nb + 1) * 128], tp[:], AF.Copy)
                    for chl in range(2):
                        for wnd in range(CHUNK_NB * 128 // 512):
                            fp = p4f.tile([128, 512], dt.float32, tag='fp', name='fp')
                            for k in range(4):
                                nc.tensor.matmul(
                                    fp[:], wout_sb[:, k, chl * 128:(chl + 1) * 128],
                                    ocT[:, k, wnd * 512:(wnd + 1) * 512],
                                    start=(k == 0), stop=(k == 3))
                            ob = p4o.tile([128, 512], dt.float32, tag='ob', name='ob')
                            nc.scalar.activation(ob[:], fp[:], AF.Identity, bias=bout_sb[:, chl:chl + 1])
                            col0 = ch * (CHUNK_NB * 128) + wnd * 512
                            nc.sync.dma_start(out_d[chl * 128:(chl + 1) * 128, col0:col0 + 512], ob[:])

    nc.compile()
    return nc


def _get_program():
    global _PROGRAM
    if _PROGRAM is None:
        import os
        _PROGRAM = _build_program(stage=int(os.environ.get('KSTAGE', '4')))
    return _PROGRAM


def kernel(x1, x2, x3, Wv, Woff1, boff1, Woff2, boff2, Waw1, baw1, Waw2, baw2, Wout, bout):
    from concourse.bass_utils import run_bass_kernel_spmd

    x1 = np.asarray(x1, dtype=np.float32)
    x2 = np.asarray(x2, dtype=np.float32)
    x3 = np.asarray(x3, dtype=np.float32)
    B = x1.shape[0]
    wcat = np.ascontiguousarray(np.concatenate(
        [np.asarray(Woff1), np.asarray(Waw1), np.asarray(Woff2), np.asarray(Waw2)],
        axis=1).astype(np.float32))
    bcat = np.ascontiguousarray(np.concatenate(
        [np.asarray(boff1), np.asarray(baw1), np.asarray(boff2), np.asarray(baw2)]
    ).astype(np.float32))
    wv = np.ascontiguousarray(np.asarray(Wv, dtype=np.float32))
    wout = np.ascontiguousarray(np.asarray(Wout, dtype=np.float32))
    boutv = np.ascontiguousarray(np.asarray(bout, dtype=np.float32))
    nx = np.arange(128, dtype=np.float32)

    nc = _get_program()
    in_maps = []
    for core in range(8):
        b, half = core // 2, core % 2
        x1f = x1[b].reshape(C, N)
        x2f = x2[b].reshape(C, N)
        in_maps.append({
            'x1h': np.ascontiguousarray(x1f[:, half * NQ:(half + 1) * NQ]),
            'x2h': np.ascontiguousarray(x2f[:, half * NQ:(half + 1) * NQ]),
            'x3f': np.ascontiguousarray(x3[b].reshape(C, N)),
            'wv': wv, 'wcat': wcat, 'bcat': bcat,
            'wout': wout, 'bout': boutv,
            'nx': nx,
            'ny': (half * NT + np.arange(NT)).astype(np.float32),
        })
    import os as _os
    ncores = int(_os.environ.get('NCORES', '8'))
    res = run_bass_kernel_spmd(nc, in_maps[:ncores], core_ids=list(range(ncores)))
    global LAST_RESULT
    LAST_RESULT = res
    out = np.zeros((B, C, HH, WW), dtype=np.float32)
    for core in range(ncores):
        b, half = core // 2, core % 2
        out[b].reshape(C, N)[:, half * NQ:(half + 1) * NQ] = res.results[core]['out']
    return out


# revision 22
# speedup vs baseline: 1.2136x; 1.0308x over previous
"""DualQDeformableAttention Trainium2 kernel (v2).

Sharding: 8 cores = 4 batch elements x 2 query-halves. Each core computes the
full dual-branch deformable attention for its 8192 queries of its batch
element (building the full value table for that batch) and emits
out[b][:, half] in C-major layout.

Per-core pipeline:
  P1: v = x3f @ Wv with a second matmul over a one-column-shifted window ->
      cell-major quad table qtab[cell][h][yy*2+xx][d] (bf16) assembled in
      SBUF (4 strided engine copies per row) and written with fully
      contiguous 256KB DMAs.
  P2: offset/attention-weight matmuls -> per-sample bilinear slot weights
      (softmax folded in, bf16) and f32 cell indices; a PE-transpose fold
      chain rearranges indices into the SWDGE wrapped-16 layout (no tiny-
      descriptor DMA), staged to DRAM in 4KB-run writes.
  P3: per (chunk, branch, head): SWDGE dma_gather with elem_step (per-head
      byte offset into the cell-major table; one 256B descriptor fetches all
      4 bilinear corners) -> bf16 weighted multiply -> strided 16:1 reduce ->
      outcat[n, (br,h,d)].  Gathers rotate across 4 SWDGE queues so
      descriptor generation runs on all four Q7 core pairs.
  P4: PE-transpose outcat, final matmul in Wout^T orientation -> C-major out.
"""

import numpy as np

HEADS = 8
NPOINTS = 4
C = 256
HH = 128
WW = 128
N = HH * WW          # 16384 cells per batch
NQ = N // 2          # 8192 queries per core
D = C // HEADS       # 32
QROW = 4 * D         # 128 elems: 4 corners x 32d per (cell, head)
CELL = HEADS * QROW  # 1024 elems per cell block
NT = NQ // 128       # 64 n-tiles per core
SLAB = 16            # n-tiles per weight-compute slab
CHUNK_NB = 8         # n-tiles per gather/blend chunk
NCHUNK = NT // CHUNK_NB   # 8
GPC = CHUNK_NB * NPOINTS  # gather groups per (chunk,br,h) = 32
IDXPC = GPC * 128         # indices per (chunk,br,h) = 4096

_PROGRAM = None
LAST_RESULT = None


def _build_program(stage=4):
    import os
    k3max = int(os.environ.get('K3CALLS', '999'))
    nqueues = int(os.environ.get('KQUEUES', '4'))
    scratch = int(os.environ.get('KSCRATCH', '32768'))
    _k3 = [0]
    import concourse.bass as bass
    import concourse.mybir as mybir
    from concourse import bacc
    from concourse.tile import TileContext
    from concourse.masks import make_identity

    dt = mybir.dt
    Alu = mybir.AluOpType
    AF = mybir.ActivationFunctionType
    AP = bass.AP
    X = mybir.AxisListType.X

    nc = bacc.Bacc('TRN2', num_swdge_queues=nqueues,
                   dynamic_dma_scratch_size=scratch)

    x1h = nc.dram_tensor('x1h', [C, NQ], dt.float32, kind='ExternalInput')
    x2h = nc.dram_tensor('x2h', [C, NQ], dt.float32, kind='ExternalInput')
    x3f = nc.dram_tensor('x3f', [C, N], dt.float32, kind='ExternalInput')
    wv_d = nc.dram_tensor('wv', [C, C], dt.float32, kind='ExternalInput')
    wcat_d = nc.dram_tensor('wcat', [C, 192], dt.float32, kind='ExternalInput')
    bcat_d = nc.dram_tensor('bcat', [192], dt.float32, kind='ExternalInput')
    wout_d = nc.dram_tensor('wout', [2 * C, C], dt.float32, kind='ExternalInput')
    bout_d = nc.dram_tensor('bout', [C], dt.float32, kind='ExternalInput')
    nx_d = nc.dram_tensor('nx', [128], dt.float32, kind='ExternalInput')
    ny_d = nc.dram_tensor('ny', [NT], dt.float32, kind='ExternalInput')
    out_d = nc.dram_tensor('out', [C, NQ], dt.float32, kind='ExternalOutput')

    W16_BR = HEADS * NT * NPOINTS * 4
    W16_H = NT * NPOINTS * 4

    with TileContext(nc) as tc:
        with tc.tile_pool(name='dram', bufs=1, space='DRAM') as dpool, \
             tc.tile_pool(name='consts', bufs=1) as cpool:

            qtab = dpool.tile([N, CELL], dt.bfloat16)
            # stag[ch][s][br][h][256]: wrapped-16 idx staging
            stag = dpool.tile([NCHUNK, 16, 2, HEADS, 256], dt.int16)

            # --- constants ---
            wv_sb = cpool.tile([128, 2, C], dt.float32)
            nc.sync.dma_start(wv_sb[:], wv_d[:].rearrange("(a p) n -> p a n", p=128))
            wcat_sb = cpool.tile([128, 2, 192], dt.float32)
            nc.sync.dma_start(wcat_sb[:], wcat_d[:].rearrange("(a p) n -> p a n", p=128))
            bcat_sb = cpool.tile([128, 192], dt.float32)
            nc.sync.dma_start(bcat_sb[:], AP(tensor=bcat_d, offset=0, ap=[[0, 128], [1, 192]]))
            wout_sb = cpool.tile([128, 4, C], dt.float32)
            nc.sync.dma_start(wout_sb[:], wout_d[:].rearrange("(a p) n -> p a n", p=128))
            bout_sb = cpool.tile([128, 2], dt.float32)
            nc.sync.dma_start(bout_sb[:], bout_d[:].rearrange("(a p) -> p a", p=128))
            nx_sb = cpool.tile([128, 1], dt.float32)
            nc.sync.dma_start(nx_sb[:], nx_d[:].rearrange("(a p) -> p a", p=128))
            ny_sb = cpool.tile([128, NT], dt.float32)
            nc.sync.dma_start(ny_sb[:], AP(tensor=ny_d, offset=0, ap=[[0, 128], [1, NT]]))
            ident = cpool.tile([128, 128], dt.float32)
            make_identity(nc, ident[:])
            identb = cpool.tile([128, 128], dt.bfloat16)
            nc.vector.tensor_copy(identb[:], ident[:])
            w16 = cpool.tile([128, 2, HEADS, NT, NPOINTS, 4], dt.bfloat16)

            # ---------------- P1: cell-major quad table ----------------
            with tc.tile_pool(name='p1x', bufs=3) as p1x, \
                 tc.tile_pool(name='p1ps', bufs=3, space='PSUM') as p1ps, \
                 tc.tile_pool(name='p1q', bufs=3) as p1q:

                def qrow_write(t, vp0, vs0, vp1, vs1):
                    qrow = p1q.tile([128, CELL], dt.bfloat16, tag='qr', name='qr')
                    for slot, src in ((0, vp0), (1, vs0), (2, vp1), (3, vs1)):
                        dst = AP(tensor=qrow.tensor, offset=qrow.offset + slot * D,
                                 ap=[qrow.ap[0], [QROW, HEADS], [1, D]])
                        srca = AP(tensor=src.tensor, offset=src.offset,
                                  ap=[src.ap[0], [D, HEADS], [1, D]])
                        if slot < 2:
                            nc.vector.tensor_copy(dst, srca)
                        else:
                            nc.scalar.activation(dst, srca, AF.Copy)
                    nc.sync.dma_start(qtab[t * 128:(t + 1) * 128, :], qrow[:])

                prev = None
                for t in range(HH):
                    xa = p1x.tile([128, 129], dt.float32, tag='xa', name='xa')
                    xb = p1x.tile([128, 129], dt.float32, tag='xb', name='xb')
                    c0 = t * 128
                    if t < HH - 1:
                        nc.sync.dma_start(xa[:], x3f[0:128, c0:c0 + 129])
                        nc.sync.dma_start(xb[:], x3f[128:256, c0:c0 + 129])
                    else:
                        nc.sync.dma_start(xa[:, 0:128], x3f[0:128, c0:c0 + 128])
                        nc.sync.dma_start(xb[:, 0:128], x3f[128:256, c0:c0 + 128])
                        nc.sync.dma_start(xa[:, 128:129], x3f[0:128, N - 1:N])
                        nc.sync.dma_start(xb[:, 128:129], x3f[128:256, N - 1:N])
                    vp = p1ps.tile([128, C], dt.float32, tag='vp', name='vp')
                    nc.tensor.matmul(vp[:], xa[:, 0:128], wv_sb[:, 0], start=True, stop=False)
                    nc.tensor.matmul(vp[:], xb[:, 0:128], wv_sb[:, 1], start=False, stop=True)
                    vs = p1ps.tile([128, C], dt.float32, tag='vs', name='vs')
                    nc.tensor.matmul(vs[:], xa[:, 1:129], wv_sb[:, 0], start=True, stop=False)
                    nc.tensor.matmul(vs[:], xb[:, 1:129], wv_sb[:, 1], start=False, stop=True)
                    if prev is not None:
                        qrow_write(t - 1, prev[0], prev[1], vp, vs)
                    prev = (vp, vs)
                qrow_write(HH - 1, prev[0], prev[1], prev[0], prev[1])

            # ---------------- P2: offsets / weights / wrapped indices ----------------
            with tc.tile_pool(name='p2x', bufs=2) as p2x, \
                 tc.tile_pool(name='p2ps', bufs=2, space='PSUM') as p2ps, \
                 tc.tile_pool(name='p2s', bufs=2) as p2s, \
                 tc.tile_pool(name='p2t', bufs=1) as p2t, \
                 tc.tile_pool(name='p2w', bufs=1) as p2w, \
                 tc.tile_pool(name='fps', bufs=2, space='PSUM') as fps, \
                 tc.tile_pool(name='fsb', bufs=2) as fsb:
                for sl in range(NT // SLAB if stage >= 2 else 0):
                    oslab = p2s.tile([128, SLAB, 192], dt.float32, name='oslab')
                    xs = []
                    for j in range(SLAB):
                        if j % 4 == 0:
                            xs = []
                            cb = sl * SLAB * 128 + (j // 4) * 512
                            for i, (src, r0, r1) in enumerate([(x1h, 0, 128), (x1h, 128, 256),
                                                               (x2h, 0, 128), (x2h, 128, 256)]):
                                xt = p2x.tile([128, 512], dt.float32, tag=f'x{i}', name=f'x{i}')
                                nc.sync.dma_start(xt[:], src[r0:r1, cb:cb + 512])
                                xs.append(xt)
                        cs = slice((j % 4) * 128, (j % 4) * 128 + 128)
                        ops = p2ps.tile([128, 192], dt.float32, tag='ops', name='ops')
                        nc.tensor.matmul(ops[:, 0:96], xs[0][:, cs], wcat_sb[:, 0, 0:96], start=True, stop=False)
                        nc.tensor.matmul(ops[:, 0:96], xs[1][:, cs], wcat_sb[:, 1, 0:96], start=False, stop=True)
                        nc.tensor.matmul(ops[:, 96:192], xs[2][:, cs], wcat_sb[:, 0, 96:192], start=True, stop=False)
                        nc.tensor.matmul(ops[:, 96:192], xs[3][:, cs], wcat_sb[:, 1, 96:192], start=False, stop=True)
                        nc.vector.tensor_tensor(out=oslab[:, j], in0=ops[:], in1=bcat_sb[:], op=Alu.add)

                    idxw = p2w.tile([16, 2, 2, HEADS, 256], dt.int16, tag='idxw', name='idxw')
                    for br in range(2):
                        base = br * 96

                        def tl(tag, shape=None, dtp=dt.float32):
                            return p2t.tile(shape or [128, SLAB, 32], dtp, tag=tag, name=tag)

                        # softmax over the 32 (h,p) logits, per query
                        esl = tl('esl')
                        aw_in = AP(tensor=oslab.tensor, offset=oslab.offset + base + 64,
                                   ap=[oslab.ap[0], [192, SLAB], [1, 32]])
                        nc.scalar.activation(esl[:], aw_in, AF.Exp)
                        ssum = tl('ssum', [128, SLAB])
                        nc.vector.tensor_reduce(op=Alu.add, out=ssum[:], in_=esl[:], axis=X)
                        sinv = tl('sinv', [128, SLAB])
                        nc.vector.reciprocal(sinv[:], ssum[:])
                        es = tl('es')
                        nc.vector.tensor_tensor(
                            out=es[:], in0=esl[:],
                            in1=AP(tensor=sinv.tensor, offset=sinv.offset,
                                   ap=[sinv.ap[0], [1, SLAB], [0, 32]]),
                            op=Alu.mult)

                        # pixel coords: ix = off_x*128 + nx ; iy = off_y*128 + ny[t]
                        offx = AP(tensor=oslab.tensor, offset=oslab.offset + base,
                                  ap=[oslab.ap[0], [192, SLAB], [2, 32]])
                        offy = AP(tensor=oslab.tensor, offset=oslab.offset + base + 1,
                                  ap=[oslab.ap[0], [192, SLAB], [2, 32]])
                        ix = tl('ix')
                        nc.vector.scalar_tensor_tensor(
                            out=ix[:], in0=offx, scalar=128.0,
                            in1=AP(tensor=nx_sb.tensor, offset=nx_sb.offset,
                                   ap=[nx_sb.ap[0], [0, SLAB], [0, 32]]),
                            op0=Alu.mult, op1=Alu.add)
                        iy = tl('iy')
                        nc.vector.scalar_tensor_tensor(
                            out=iy[:], in0=offy, scalar=128.0,
                            in1=AP(tensor=ny_sb.tensor, offset=ny_sb.offset + sl * SLAB,
                                   ap=[ny_sb.ap[0], [1, SLAB], [0, 32]]),
                            op0=Alu.mult, op1=Alu.add)

                        def floorfrac(coord, pfx):
                            # floor via round(x - 0.5): f32->i32 convert rounds
                            half = tl(pfx + 'h')
                            nc.vector.tensor_scalar(out=half[:], in0=coord[:],
                                                    scalar1=-0.5, scalar2=None, op0=Alu.add)
                            ci = tl(pfx + 'i', dtp=dt.int32)
                            nc.vector.tensor_copy(ci[:], half[:])
                            cf = tl(pfx + 'f')
                            nc.vector.tensor_copy(cf[:], ci[:])
                            fr = tl(pfx + 'r')
                            nc.vector.tensor_tensor(out=fr[:], in0=coord[:], in1=cf[:], op=Alu.subtract)
                            return cf, fr

                        x0f, fx = floorfrac(ix, 'fx')
                        y0f, fy = floorfrac(iy, 'fy')

                        def slotw(c0f, fr, pfx):
                            # s0 = (1-f)*[0<=c0<=126] + f*[c0==-1]
                            # s1 = f*[0<=c0<=126] + (1-f)*[c0==127]
                            ge = tl(pfx + 'ge')
                            nc.vector.tensor_scalar(out=ge[:], in0=c0f[:], scalar1=0.0, scalar2=None, op0=Alu.is_ge)
                            le = tl(pfx + 'le')
                            nc.vector.tensor_scalar(out=le[:], in0=c0f[:], scalar1=126.0, scalar2=None, op0=Alu.is_le)
                            ax = tl(pfx + 'ax')
                            nc.vector.tensor_tensor(out=ax[:], in0=ge[:], in1=le[:], op=Alu.mult)
                            blo = tl(pfx + 'blo')
                            nc.vector.tensor_scalar(out=blo[:], in0=c0f[:], scalar1=-1.0, scalar2=None, op0=Alu.is_equal)
                            bhi = tl(pfx + 'bhi')
                            nc.vector.tensor_scalar(out=bhi[:], in0=c0f[:], scalar1=127.0, scalar2=None, op0=Alu.is_equal)
                            omf = tl(pfx + 'omf')
                            nc.vector.tensor_scalar(out=omf[:], in0=fr[:], scalar1=-1.0, scalar2=1.0, op0=Alu.mult, op1=Alu.add)
                            s0 = tl(pfx + 's0')
                            nc.vector.tensor_tensor(out=s0[:], in0=omf[:], in1=ax[:], op=Alu.mult)
                            t0 = tl(pfx + 't0')
                            nc.vector.tensor_tensor(out=t0[:], in0=fr[:], in1=blo[:], op=Alu.mult)
                            nc.vector.tensor_tensor(out=s0[:], in0=s0[:], in1=t0[:], op=Alu.add)
                            s1 = tl(pfx + 's1')
                            nc.vector.tensor_tensor(out=s1[:], in0=fr[:], in1=ax[:], op=Alu.mult)
                            t1 = tl(pfx + 't1')
                            nc.vector.tensor_tensor(out=t1[:], in0=omf[:], in1=bhi[:], op=Alu.mult)
                            nc.vector.tensor_tensor(out=s1[:], in0=s1[:], in1=t1[:], op=Alu.add)
                            return s0, s1

                        sx0, sx1 = slotw(x0f, fx, 'sx')
                        sy0, sy1 = slotw(y0f, fy, 'sy')

                        ay0 = tl('ay0')
                        nc.vector.tensor_tensor(out=ay0[:], in0=sy0[:], in1=es[:], op=Alu.mult)
                        ay1 = tl('ay1')
                        nc.vector.tensor_tensor(out=ay1[:], in0=sy1[:], in1=es[:], op=Alu.mult)

                        for (qi, ayv, sxv) in ((0, ay0, sx0), (1, ay0, sx1),
                                               (2, ay1, sx0), (3, ay1, sx1)):
                            wdst = AP(tensor=w16.tensor,
                                      offset=w16.offset + br * W16_BR + sl * SLAB * (NPOINTS * 4) + qi,
                                      ap=[w16.ap[0], [NPOINTS * 4, SLAB], [W16_H, HEADS], [4, NPOINTS]])
                            win0 = AP(tensor=ayv.tensor, offset=ayv.offset,
                                      ap=[ayv.ap[0], [32, SLAB], [NPOINTS, HEADS], [1, NPOINTS]])
                            win1 = AP(tensor=sxv.tensor, offset=sxv.offset,
                                      ap=[sxv.ap[0], [32, SLAB], [NPOINTS, HEADS], [1, NPOINTS]])
                            nc.vector.tensor_tensor(out=wdst, in0=win0, in1=win1, op=Alu.mult)

                        xbc = tl('xb')
                        nc.vector.tensor_scalar(out=xbc[:], in0=x0f[:], scalar1=0.0, scalar2=126.0, op0=Alu.max, op1=Alu.min)
                        ybc = tl('yb')
                        nc.vector.tensor_scalar(out=ybc[:], in0=y0f[:], scalar1=0.0, scalar2=126.0, op0=Alu.max, op1=Alu.min)
                        idxf = tl('idxf')
                        nc.vector.scalar_tensor_tensor(out=idxf[:], in0=ybc[:], scalar=128.0,
                                                       in1=xbc[:], op0=Alu.mult, op1=Alu.add)
                        # idxf in h-major layout [128][h][nt][p]
                        idxft = tl('idxft', [128, HEADS, SLAB, NPOINTS])
                        nc.vector.tensor_copy(
                            idxft[:],
                            AP(tensor=idxf.tensor, offset=idxf.offset,
                               ap=[idxf.ap[0], [NPOINTS, HEADS], [32, SLAB], [1, NPOINTS]]))

                        # PE fold: [128 q][32 (nt,p)] -> wrapped [16][(g,j)]
                        for c in range(2):
                            for h in range(HEADS):
                                tp1 = fps.tile([32, 128], dt.float32, tag='tp1', name='tp1')
                                nc.tensor.transpose(tp1[:], idxft[:, h, c * 8:(c + 1) * 8, :], ident[:])
                                ts1 = fsb.tile([32, 128], dt.float32, tag='ts1', name='ts1')
                                nc.scalar.activation(ts1[:], tp1[:], AF.Copy)
                                uall = fps.tile([16, 8, 32], dt.float32, tag='u', name='u')
                                for jj in range(8):
                                    nc.tensor.transpose(uall[:, jj, :], ts1[:, 16 * jj:16 * jj + 16], ident[0:32, 0:32])
                                udst = AP(tensor=idxw.tensor,
                                          offset=idxw.offset + ((c * 2 + br) * HEADS + h) * 256,
                                          ap=[idxw.ap[0], [8, 32], [1, 8]])
                                usrc = AP(tensor=uall.tensor, offset=uall.offset,
                                          ap=[uall.ap[0], [1, 32], [32, 8]])
                                nc.vector.tensor_copy(udst, usrc)

                    for c in range(2):
                        ch = sl * 2 + c
                        nc.sync.dma_start(stag[ch], idxw[:, c])

            # ---------------- P3 + P4 ----------------
            with tc.tile_pool(name='p3i', bufs=2) as p3i, \
                 tc.tile_pool(name='p3g', bufs=6) as p3g, \
                 tc.tile_pool(name='p3w', bufs=3) as p3w, \
                 tc.tile_pool(name='p3o', bufs=2) as p3o, \
                 tc.tile_pool(name='p4ps', bufs=4, space='PSUM') as p4ps, \
                 tc.tile_pool(name='p4t', bufs=1) as p4t, \
                 tc.tile_pool(name='p4f', bufs=2, space='PSUM') as p4f, \
                 tc.tile_pool(name='p4o', bufs=2) as p4o:
                for ch in range(NCHUNK if stage >= 3 else 0):
                    idxt = p3i.tile([128, 2 * HEADS * 256], dt.int16, tag='idxt', name='idxt')
                    ssrc = AP(tensor=stag.tensor, offset=stag.offset + ch * (16 * 4096),
                              ap=[[0, 8], [4096, 16], [1, 4096]])
                    nc.sync.dma_start(idxt[:], ssrc)
                    outcat = p3o.tile([128, CHUNK_NB, 512], dt.bfloat16, name='outcat')
                    for br in range(2):
                        for h in range(HEADS):
                            if _k3[0] >= k3max:
                                continue
                            _k3[0] += 1
                            gat = p3g.tile([128, GPC, QROW], dt.bfloat16, tag='gat', name='gat')
                            qv = AP(tensor=qtab.tensor, offset=qtab.offset + h * QROW,
                                    ap=[[CELL, N], [1, QROW]])
                            ib = (br * HEADS + h) * 256
                            for sub in range(IDXPC // 1024):
                                nc.gpsimd.dma_gather(
                                    out_ap=gat[:, sub * 8:(sub + 1) * 8, :],
                                    in_ap=qv,
                                    idxs_ap=idxt[:, ib + sub * 64:ib + (sub + 1) * 64],
                                    num_idxs=1024, num_idxs_reg=1024,
                                    elem_size=QROW, elem_step=CELL,
                                    queue_num=(br * HEADS + h) % nqueues)
                            wg = p3w.tile([128, GPC, QROW], dt.bfloat16, tag='wg', name='wg')
                            win = AP(tensor=w16.tensor,
                                     offset=w16.offset + br * W16_BR + h * W16_H
                                     + ch * CHUNK_NB * NPOINTS * 4,
                                     ap=[w16.ap[0], [4, GPC], [1, 4], [0, D]])
                            nc.vector.tensor_tensor(
                                out=wg[:].rearrange("p g (q d) -> p g q d", d=D),
                                in0=gat[:].rearrange("p g (q d) -> p g q d", d=D),
                                in1=win, op=Alu.mult)
                            rin = AP(tensor=wg.tensor, offset=wg.offset,
                                     ap=[wg.ap[0], [NPOINTS * QROW, CHUNK_NB], [1, D], [D, 16]])
                            rout = AP(tensor=outcat.tensor,
                                      offset=outcat.offset + br * 256 + h * D,
                                      ap=[outcat.ap[0], [512, CHUNK_NB], [1, D]])
                            with nc.allow_low_precision(reason='16:1 corner sum fits bf16; 2e-2 gate'):
                                nc.vector.tensor_reduce(op=Alu.add, out=rout, in_=rin, axis=X)

                    if stage < 4:
                        continue
                    ocT = p4t.tile([128, 4, CHUNK_NB * 128], dt.float32, name='ocT')
                    for nb in range(CHUNK_NB):
                        for k in range(4):
                            tp = p4ps.tile([128, 128], dt.bfloat16, tag='tp', name='tp')
                            nc.tensor.transpose(tp[:], outcat[:, nb, k * 128:(k + 1) * 128], identb[:])
                            nc.scalar.activation(ocT[:, k, nb * 128:(nb + 1) * 128], tp[:], AF.Copy)
                    for chl in range(2):
                        for wnd in range(CHUNK_NB * 128 // 512):
                            fp = p4f.tile([128, 512], dt.float32, tag='fp', name='fp')
                            for k in range(4):
                                nc.tensor.matmul(
                                    fp[:], wout_sb[:, k, chl * 128:(chl + 1) * 128],
                                    ocT[:, k, wnd * 512:(wnd + 1) * 512],
                                    start=(k == 0), stop=(k == 3))
                            ob = p4o.tile([128, 512], dt.float32, tag='ob', name='ob')
                            nc.scalar.activation(ob[:], fp[:], AF.Identity, bias=bout_sb[:, chl:chl + 1])
                            col0 = ch * (CHUNK_NB * 128) + wnd * 512
                            nc.sync.dma_start(out_d[chl * 128:(chl + 1) * 128, col0:col0 + 512], ob[:])

    nc.compile()
    return nc


def _get_program():
    global _PROGRAM
    if _PROGRAM is None:
        import os
        _PROGRAM = _build_program(stage=int(os.environ.get('KSTAGE', '4')))
    return _PROGRAM


def kernel(x1, x2, x3, Wv, Woff1, boff1, Woff2, boff2, Waw1, baw1, Waw2, baw2, Wout, bout):
    from concourse.bass_utils import run_bass_kernel_spmd

    x1 = np.asarray(x1, dtype=np.float32)
    x2 = np.asarray(x2, dtype=np.float32)
    x3 = np.asarray(x3, dtype=np.float32)
    B = x1.shape[0]
    wcat = np.ascontiguousarray(np.concatenate(
        [np.asarray(Woff1), np.asarray(Waw1), np.asarray(Woff2), np.asarray(Waw2)],
        axis=1).astype(np.float32))
    bcat = np.ascontiguousarray(np.concatenate(
        [np.asarray(boff1), np.asarray(baw1), np.asarray(boff2), np.asarray(baw2)]
    ).astype(np.float32))
    wv = np.ascontiguousarray(np.asarray(Wv, dtype=np.float32))
    wout = np.ascontiguousarray(np.asarray(Wout, dtype=np.float32))
    boutv = np.ascontiguousarray(np.asarray(bout, dtype=np.float32))
    nx = np.arange(128, dtype=np.float32)

    nc = _get_program()
    in_maps = []
    for core in range(8):
        b, half = core // 2, core % 2
        x1f = x1[b].reshape(C, N)
        x2f = x2[b].reshape(C, N)
        in_maps.append({
            'x1h': np.ascontiguousarray(x1f[:, half * NQ:(half + 1) * NQ]),
            'x2h': np.ascontiguousarray(x2f[:, half * NQ:(half + 1) * NQ]),
            'x3f': np.ascontiguousarray(x3[b].reshape(C, N)),
            'wv': wv, 'wcat': wcat, 'bcat': bcat,
            'wout': wout, 'bout': boutv,
            'nx': nx,
            'ny': (half * NT + np.arange(NT)).astype(np.float32),
        })
    import os as _os
    ncores = int(_os.environ.get('NCORES', '8'))
    res = run_bass_kernel_spmd(nc, in_maps[:ncores], core_ids=list(range(ncores)))
    global LAST_RESULT
    LAST_RESULT = res
    out = np.zeros((B, C, HH, WW), dtype=np.float32)
    for core in range(ncores):
        b, half = core // 2, core % 2
        out[b].reshape(C, N)[:, half * NQ:(half + 1) * NQ] = res.results[core]['out']
    return out
